# revision 1
# baseline (speedup 1.0000x reference)
"""Trainium2 Bass kernel for nn_AutoregressiveSplineDeep (autoregressive
linear-rational spline flow, D=2, K=16, H=128, flow_length=8).

Self-contained: hardcodes problem shapes; computes all derived constants from
the runtime inputs; shards batch across 8 NeuronCores (pure data parallel).

Math summary (derived from the MADE masks for D=2):
 - dim-0 spline params are constant (= b2 even rows): the dim-0 transform is a
   fixed piecewise-Moebius map with 32 pieces; host precomputes coefficient
   tables in fp64.
 - dim-1 spline params depend only on x0: with b0=b1=0 the hypernet collapses
   to params63 = x0+ * qp + x0- * qn + b2_odd (rank-2, one K=2 matmul);
   otherwise the general two-matmul MLP path is used.
 - bin search is a compare against the unnormalized running softmax sum
   (masked segmented scan); gathers are onehot dot-products; the rational is
   rescaled to need a single divide.
"""
import os, sys
for _p in ('/opt/trn_rl_repo', '/root/.axon_site/_ro/trn_rl_repo'):
    if os.path.isdir(_p) and _p not in sys.path:
        sys.path.insert(0, _p)
        break

import numpy as np

D, K, H = 2, 16, 128
BOUND = 5.0
FLOW_LEN = 8
N_FULL = 524288
N_CORES = 8
MB = 1e-3
MD = 1e-3
C1 = 1.0 - MB * K
MAGIC = 12582912.0  # 1.5 * 2**23, fp32 round-to-nearest-int trick

f32 = np.float32


def _np_softmax(x):
    e = np.exp(x - x.max())
    return e / e.sum()


def _np_softplus(x):
    return np.log1p(np.exp(-np.abs(x))) + np.maximum(x, 0)


def _np_sigmoid(x):
    return 1.0 / (1.0 + np.exp(-x))


def precompute(W0, b0, W1, b1, W2, b2):
    """fp64 host-side precompute of all derived constants."""
    W0, W1, W2 = (a.astype(np.float64) for a in (W0, W1, W2))
    b0, b1, b2 = (a.astype(np.float64) for a in (b0, b1, b2))
    a = W0[:, 0]
    u_p = W1 @ np.maximum(a, 0)
    u_n = W1 @ np.minimum(a, 0)
    W2odd = W2[1::2, :]
    b2odd = b2[1::2]
    qp = W2odd @ np.maximum(u_p, 0)
    qn = W2odd @ np.minimum(u_n, 0)
    fast_hyper = bool(np.all(b0 == 0) and np.all(b1 == 0))
    b2zero = bool(np.all(b2odd == 0))

    p0 = b2[0::2]
    w0, h0 = p0[:K], p0[K:2 * K]
    d0, l0 = p0[2 * K:3 * K - 1], p0[3 * K - 1:]
    widths = MB + C1 * _np_softmax(w0)
    cw = np.concatenate([[0.0], np.cumsum(widths)]) * (2 * BOUND) - BOUND
    cw[0], cw[-1] = -BOUND, BOUND
    widths = np.diff(cw)
    heights = MB + C1 * _np_softmax(h0)
    ch = np.concatenate([[0.0], np.cumsum(heights)]) * (2 * BOUND) - BOUND
    ch[0], ch[-1] = -BOUND, BOUND
    heights = np.diff(ch)
    delta = heights / widths
    dv = np.concatenate([[1.0], MD + _np_softplus(d0), [1.0]])
    lam = 0.95 * _np_sigmoid(l0) + 0.025

    uniform32 = bool(np.allclose(w0, w0[0]) and np.allclose(l0, l0[0])
                     and abs(_np_sigmoid(l0[0]) - 0.5) < 1e-12)
    A32 = np.zeros(32); B32 = np.zeros(32); G32 = np.zeros(32); D32 = np.zeros(32)
    bnd = np.zeros(32)  # piece start boundaries (piece p covers [bnd_p, bnd_{p+1}))
    for k in range(K):
        xk, wk = cw[k], widths[k]
        yk, hk = ch[k], heights[k]
        dk, dk1 = dv[k], dv[k + 1]
        lk = lam[k]
        wb = np.sqrt(dk / dk1)
        wc = (lk * dk + (1 - lk) * wb * dk1) / delta[k]
        ya, yb = yk, yk + hk
        yc = ((1 - lk) * ya + lk * wb * yb) / ((1 - lk) + lk * wb)
        a_l = ya * (lk * wk + xk) - wc * yc * xk
        b_l = -ya + wc * yc
        g_l = (lk * wk + xk) - wc * xk
        d_l = -1.0 + wc
        a_r = wc * yc * (wk + xk) - wb * yb * (xk + lk * wk)
        b_r = -wc * yc + wb * yb
        g_r = wc * (wk + xk) - wb * (xk + lk * wk)
        d_r = -wc + wb
        A32[2 * k:2 * k + 2] = a_l, a_r
        B32[2 * k:2 * k + 2] = b_l, b_r
        G32[2 * k:2 * k + 2] = g_l, g_r
        D32[2 * k:2 * k + 2] = d_l, d_r
        bnd[2 * k] = cw[k]
        bnd[2 * k + 1] = cw[k] + lk * wk
    An, Bn, Dn = A32 / G32, B32 / G32, D32 / G32
    return dict(
        u2=np.stack([u_p, u_n]).astype(f32),            # (2,128)
        b1c=b1.astype(f32).reshape(H, 1),
        w2t=np.ascontiguousarray(W2odd.T).astype(f32),  # (128,63)
        qb=np.stack([qp, qn]).astype(f32),              # (2,63)
        b2odd=b2odd.astype(f32),
        fast_hyper=fast_hyper, b2zero=b2zero, uniform32=uniform32,
        tabA=An.astype(f32), tabB=Bn.astype(f32), tabD=Dn.astype(f32),
        bnd32=bnd.astype(f32),
    )


# ----------------------------------------------------------------------------
# program builder
# ----------------------------------------------------------------------------
def build_program(pc, ncols=512, cchunk=64, nsteps=FLOW_LEN, cfg=None):
    import concourse.bass as bass
    import concourse.tile as tile
    from concourse import bacc, mybir
    from concourse.masks import make_identity
    from contextlib import ExitStack

    FP = mybir.dt.float32
    U8 = mybir.dt.uint8
    AL = mybir.AluOpType
    AF = mybir.ActivationFunctionType
    nsamp = 128 * ncols
    CC = cchunk
    nchunk = ncols // CC
    nch2 = nchunk // 2
    WH = ncols // 2          # half width
    assert ncols % CC == 0 and nchunk % 2 == 0

    nc = bacc.Bacc('TRN2', target_bir_lowering=False, debug=False)

    z_ap = nc.dram_tensor('z', [nsamp, D], FP, kind='ExternalInput').ap()
    y_ap = nc.dram_tensor('y', [nsamp, D], FP, kind='ExternalOutput').ap()
    qb_ap = nc.dram_tensor('qb', [2, 63], FP, kind='ExternalInput').ap()
    u2_ap = nc.dram_tensor('u2', [2, H], FP, kind='ExternalInput').ap()
    w2t_ap = nc.dram_tensor('w2t', [H, 63], FP, kind='ExternalInput').ap()
    b1c_ap = nc.dram_tensor('b1c', [H, 1], FP, kind='ExternalInput').ap()
    crep_ap = nc.dram_tensor('crep', [128, 16 + 32 + 32 + 96 + 512 + CC * 16],
                             FP, kind='ExternalInput').ap()

    # p-major sample mapping: sample s = p*ncols + f (contiguous 4KB/partition)
    zr = z_ap.rearrange('(p f) d -> p f d', p=128)
    yr = y_ap.rearrange('(p f) d -> p f d', p=128)

    with tile.TileContext(nc) as tc, ExitStack() as octx:
        const_pool = octx.enter_context(tc.tile_pool(name='const', bufs=1))
        state_pool = octx.enter_context(tc.tile_pool(name='state', bufs=1))
        work = octx.enter_context(tc.tile_pool(name='work', bufs=2))
        prodp = octx.enter_context(tc.tile_pool(name='prod', bufs=2))
        smalls = octx.enter_context(tc.tile_pool(name='smalls', bufs=1))
        psum = octx.enter_context(
            tc.tile_pool(name='psum', bufs=2, space='PSUM'))
        psumt = octx.enter_context(
            tc.tile_pool(name='psumt', bufs=2, space='PSUM'))

        # ---- constants ----
        crep = const_pool.tile([128, 16 + 32 + 32 + 96 + 512 + CC * 16], FP)
        nc.sync.dma_start(crep[:], crep_ap[:])
        kv16 = crep[:, 0:16]
        iota32 = crep[:, 16:48]
        bnd32 = crep[:, 48:80]
        tab3 = crep[:, 80:176]
        b2rep = crep[:, 176:688]
        mskpat = crep[:, 688:688 + CC * 16]
        qb = const_pool.tile([2, 63], FP)
        nc.sync.dma_start(qb[:], qb_ap[:])
        u2 = const_pool.tile([2, H], FP)
        nc.sync.dma_start(u2[:], u2_ap[:])
        w2t = const_pool.tile([H, 63], FP)
        nc.sync.dma_start(w2t[:], w2t_ap[:])
        b1c = const_pool.tile([H, 1], FP)
        nc.sync.dma_start(b1c[:], b1c_ap[:])
        ident = const_pool.tile([128, 128], FP)
        make_identity(nc, ident[:])
        c025 = const_pool.tile([128, 1], FP)
        nc.vector.memset(c025[:], 0.025)
        ones1 = const_pool.tile([128, 1], FP)
        nc.vector.memset(ones1[:], 1.0)

        # ---- states: per (buf, dim, half) ----
        xs = [[[state_pool.tile([128, WH], FP, name='x_%d_%d_%d' % (b, d_, hh))
                for hh in range(2)] for d_ in range(2)] for b in range(2)]
        zin = state_pool.tile([128, ncols, D], FP)
        nc.sync.dma_start(zin[:], zr[:])
        for hh in range(2):
            hsl = slice(hh * WH, (hh + 1) * WH)
            nc.scalar.copy(xs[0][0][hh][:], zin[:, hsl, 0])
            nc.scalar.copy(xs[0][1][hh][:], zin[:, hsl, 1])
        yout = state_pool.tile([128, ncols, D], FP)

        def bc(ap, shape):
            return ap.broadcast_to(shape)

        V = nc.vector
        G = nc.gpsimd
        A = nc.scalar

        for step in range(nsteps):
            last = (step == nsteps - 1)
            for hh in range(2):
                x0r, x1r = xs[step % 2][0][hh], xs[step % 2][1][hh]
                if last:
                    hsl = slice(hh * WH, (hh + 1) * WH)
                    x0w, x1w = yout[:, hsl, 0], yout[:, hsl, 1]
                else:
                    x0w, x1w = (xs[(step + 1) % 2][0][hh][:],
                                xs[(step + 1) % 2][1][hh][:])
                SW = (128, WH)

                def ftile(tag, dt=FP):
                    return smalls.tile([128, WH], dt, tag=tag + str(hh),
                                       name=tag + str(hh))

                # ---- prologue ----
                xc1 = ftile('xc1')
                V.tensor_scalar(xc1[:], x1r[:], float(BOUND), -float(BOUND),
                                AL.min, AL.max)
                Apr = ftile('Apr')
                V.tensor_scalar(Apr[:], xc1[:], float(1 / (2 * BOUND * C1)),
                                float(5 / (2 * BOUND * C1)), AL.mult, AL.add)
                xc0 = ftile('xc0')
                V.tensor_scalar(xc0[:], x0r[:], float(BOUND), -float(BOUND),
                                AL.min, AL.max)
                idx0 = ftile('idx0')
                if pc['uniform32']:
                    V.tensor_scalar(idx0[:], xc0[:], 3.2, 15.5,
                                    AL.mult, AL.add)
                    V.tensor_scalar(idx0[:], idx0[:], MAGIC, MAGIC,
                                    AL.add, AL.subtract)
                    V.tensor_scalar(idx0[:], idx0[:], 0.0, 31.0,
                                    AL.max, AL.min)

                sfu = ftile('sfu'); shf = ftile('shf')
                gcpf = ftile('gcpf'); gchpf = ftile('gchpf')
                g4f = smalls.tile([128, WH, 4], FP, tag='g4f' + str(hh),
                                  name='g4f' + str(hh))
                gpdm1f = ftile('gpdm1f'); idxf = ftile('idxf')
                g3f = smalls.tile([128, WH, 3], FP, tag='g3f' + str(hh),
                                  name='g3f' + str(hh))

                # ---- chunks ----
                for cl in range(nch2):
                    cs = slice(cl * CC, (cl + 1) * CC)
                    x0c = x0r[:, cs]

                    pack = smalls.tile([128, 2 * CC], FP, tag='pack',
                                       name='pack', bufs=2)
                    A.activation(pack[:, 0:CC], x0c, AF.Relu)
                    G.tensor_tensor(pack[:, CC:2 * CC], x0c, pack[:, 0:CC],
                                    AL.subtract)
                    assert 2 * CC <= 128
                    xtp = psumt.tile([128, 128], FP)
                    nc.tensor.transpose(xtp[0:2 * CC, :], pack[:], ident[:])
                    xts = smalls.tile([128, 128], FP, tag='xts', name='xts',
                                      bufs=3)
                    A.copy(xts[0:2 * CC, :], xtp[0:2 * CC, :])
                    GB = 16
                    for g in range(CC // GB):
                        pg = psum.tile([128, 2, 512], FP, bufs=3)
                        x2s = smalls.tile([2, GB * 128], FP, tag='x2s',
                                          name='x2s', bufs=2)
                        nc.sync.dma_start(x2s[0:1, :],
                                          xts[g * GB:(g + 1) * GB, :])
                        nc.sync.dma_start(
                            x2s[1:2, :], xts[CC + g * GB:CC + (g + 1) * GB, :])
                        if pc['fast_hyper']:
                            if not pc['b2zero']:
                                A.copy(pg[:], bc(b2rep.unsqueeze(1),
                                                 (128, 2, 512)))
                            for bl in range(GB):
                                lhsT = x2s[0:2, bl * 128:(bl + 1) * 128]
                                nc.tensor.matmul(
                                    pg[:, bl // 8,
                                       63 * (bl % 8):63 * (bl % 8) + 63],
                                    lhsT, qb[:], start=pc['b2zero'], stop=True)
                        else:
                            if not pc['b2zero']:
                                A.copy(pg[:], bc(b2rep.unsqueeze(1),
                                                 (128, 2, 512)))
                            for bl in range(GB):
                                pre = psumt.tile([128, 128], FP, tag='pre',
                                                 name='pre')
                                rhs = x2s[0:2, bl * 128:(bl + 1) * 128]
                                nc.tensor.matmul(pre[:], u2[:], rhs,
                                                 start=True, stop=True)
                                h1b = work.tile([128, 128], FP, tag='h1b',
                                                name='h1b')
                                A.activation(h1b[:], pre[:], AF.Relu,
                                             bias=b1c[:])
                                nc.tensor.matmul(
                                    pg[:, bl // 8,
                                       63 * (bl % 8):63 * (bl % 8) + 63],
                                    h1b[:], w2t[:], start=pc['b2zero'],
                                    stop=True)
                        ls = slice(g * GB, (g + 1) * GB)
                        srcp = pg[:, :, 0:504].rearrange(
                            'p b (k j) -> p b k j', j=63)
                        if g == 0:
                            ew = work.tile([128, CC, 16], FP, tag='ew',
                                           name='ew', bufs=2)
                            pk4 = work.tile([128, CC, 4, 16], FP, tag='pk4',
                                            name='pk4', bufs=1)
                            V.memset(pk4[:, :, 2, 15], 0.0)
                        for dst, lo, hi, fn in (
                                (ew[:, ls, :], 0, 16, AF.Exp),
                                (pk4[:, ls, 0, :], 0, 16, AF.Exp),
                                (pk4[:, ls, 1, :], 16, 32, AF.Exp),
                                (pk4[:, ls, 2, 0:15], 32, 47, AF.Copy),
                                (pk4[:, ls, 3, :], 47, 63, AF.Copy)):
                            A.activation(
                                dst.rearrange('p (a b) k -> p a b k', a=2),
                                srcp[:, :, :, lo:hi], fn)

                    T3 = (128, CC, 16)
                    cum = work.tile(list(T3), FP, tag='cum', name='cum',
                                    bufs=1)
                    V.tensor_tensor_scan(
                        cum[:].rearrange('p c k -> p (c k)'), mskpat,
                        ew[:].rearrange('p c k -> p (c k)'), 0.0,
                        AL.mult, AL.add)
                    sw = cum[:, :, 15]
                    V.tensor_scalar(sfu[:, cs], sw, float(MB / C1), None,
                                    AL.mult)
                    r1 = smalls.tile([128, CC], FP, tag='r1', name='r1',
                                     bufs=2)
                    V.tensor_tensor(r1[:], Apr[:, cs], sw, AL.mult)
                    ks = work.tile(list(T3), FP, tag='ks', name='ks', bufs=1)
                    G.tensor_tensor(ks[:], bc(kv16.unsqueeze(1), T3),
                                    bc(sfu[:, cs].unsqueeze(2), T3), AL.mult)
                    ut = work.tile(list(T3), FP, tag='ut', name='ut', bufs=1)
                    V.tensor_tensor(ut[:], cum[:], ks[:], AL.add)
                    mt = work.tile(list(T3), FP, tag='mt', name='mt', bufs=1)
                    V.tensor_tensor(mt[:], bc(r1[:].unsqueeze(2), T3), ut[:],
                                    AL.is_ge)
                    oh = work.tile(list(T3), FP, tag='oh', name='oh', bufs=1)
                    V.tensor_scalar(oh[:, :, 0], mt[:, :, 0], -1.0, 1.0,
                                    AL.mult, AL.add)
                    G.tensor_tensor(oh[:, :, 1:15], mt[:, :, 0:14],
                                    mt[:, :, 1:15], AL.subtract)
                    V.tensor_copy(oh[:, :, 15], mt[:, :, 14])
                    V.tensor_reduce(idxf[:, cs], mt[:], mybir.AxisListType.X,
                                    AL.add)
                    V.tensor_reduce(shf[:, cs], pk4[:, :, 1, :],
                                    mybir.AxisListType.X, AL.add)

                    pr2 = prodp.tile([128, CC, 2, 16], FP, tag='prm',
                                     name='pr2', bufs=1)
                    G.tensor_tensor(pr2[:],
                                    bc(mt[:].unsqueeze(2), (128, CC, 2, 16)),
                                    pk4[:, :, 0:2, :], AL.mult)
                    V.tensor_reduce(gcpf[:, cs], pr2[:, :, 0, :],
                                    mybir.AxisListType.X, AL.add)
                    V.tensor_reduce(gchpf[:, cs], pr2[:, :, 1, :],
                                    mybir.AxisListType.X, AL.add)
                    pr4 = prodp.tile([128, CC, 4, 16], FP, tag='prm',
                                     name='pr4', bufs=1)
                    G.tensor_tensor(pr4[:],
                                    bc(oh[:].unsqueeze(2), (128, CC, 4, 16)),
                                    pk4[:], AL.mult)
                    V.tensor_reduce(g4f[:, cs, :], pr4[:],
                                    mybir.AxisListType.X, AL.add)
                    prS = prodp.tile([128, CC, 15], FP, tag='prS',
                                     name='prS', bufs=1)
                    V.tensor_tensor(prS[:], oh[:, :, 1:16],
                                    pk4[:, :, 2, 0:15], AL.mult)
                    V.tensor_reduce(gpdm1f[:, cs], prS[:],
                                    mybir.AxisListType.X, AL.add)

                    T32 = (128, CC, 32)
                    if not pc['uniform32']:
                        m0 = prodp.tile([128, CC, 32], FP, tag='oh0',
                                        name='m0', bufs=1)
                        V.tensor_tensor(m0[:], bc(xc0[:, cs].unsqueeze(2),
                                                  T32),
                                        bc(bnd32.unsqueeze(1), T32), AL.is_ge)
                        V.tensor_reduce(idx0[:, cs], m0[:],
                                        mybir.AxisListType.X, AL.add)
                        V.tensor_scalar(idx0[:, cs], idx0[:, cs], 1.0, 31.0,
                                        AL.subtract, AL.min)
                    oh0 = prodp.tile([128, CC, 32], FP, tag='oh0',
                                     name='oh0', bufs=1)
                    V.tensor_tensor(oh0[:], bc(iota32.unsqueeze(1), T32),
                                    bc(idx0[:, cs].unsqueeze(2), T32),
                                    AL.is_equal)
                    pr30 = prodp.tile([128, CC, 3, 32], FP, tag='pr30',
                                      name='pr30', bufs=1)
                    G.tensor_tensor(
                        pr30[:], bc(oh0[:].unsqueeze(2), (128, CC, 3, 32)),
                        bc(tab3[:].rearrange('p (t j) -> p t j', j=32)
                           .unsqueeze(1), (128, CC, 3, 32)), AL.mult)
                    V.tensor_reduce(g3f[:, cs, :], pr30[:],
                                    mybir.AxisListType.X, AL.add)

                # ---- epilogue (half width) ----
                lndk = ftile('lndk')
                lndk1 = ftile('lndk1')
                sc0 = ftile('sc0')
                e1 = ftile('e1'); A.activation(e1[:], gpdm1f[:], AF.Exp)
                e2 = ftile('e2'); A.activation(e2[:], g4f[:, :, 2], AF.Exp)
                es = ftile('es'); A.activation(es[:], g4f[:, :, 3], AF.Exp,
                                               scale=-1.0)
                A.activation(e1[:], e1[:], AF.Ln, bias=1.0)
                A.activation(e2[:], e2[:], AF.Ln, bias=1.0)
                rw = ftile('rw')
                V.reciprocal(rw[:], sfu[:])
                rh = ftile('rh')
                V.reciprocal(rh[:], shf[:])
                V.tensor_scalar(rw[:], rw[:], float(2 * BOUND * MB), None,
                                AL.mult)
                V.tensor_scalar(rh[:], rh[:], float(2 * BOUND * C1), None,
                                AL.mult)
                dk = e1
                V.tensor_scalar(dk[:], dk[:], float(MD), None, AL.add)
                dk1 = e2
                V.tensor_scalar(dk1[:], dk1[:], float(MD), None, AL.add)
                mk0 = ftile('mk0', U8)
                V.tensor_scalar(mk0[:], idxf[:], 0.5, None, AL.is_lt)
                mk15 = ftile('mk15', U8)
                V.tensor_scalar(mk15[:], idxf[:], 14.5, None, AL.is_ge)
                V.copy_predicated(dk[:], mk0[:], bc(ones1, SW))
                V.copy_predicated(dk1[:], mk15[:], bc(ones1, SW))
                idxc = idxf
                V.tensor_scalar(idxc[:], idxc[:], float(K - 1), None, AL.min)
                A.activation(lndk[:], dk[:], AF.Ln)
                A.activation(lndk1[:], dk1[:], AF.Ln)
                ldr = lndk
                V.tensor_tensor(ldr[:], ldr[:], lndk1[:], AL.subtract)
                wb = lndk
                A.activation(wb[:], ldr[:], AF.Exp, scale=0.5)
                lk = es
                G.tensor_scalar(lk[:], lk[:], 1.0, None, AL.add)
                V.reciprocal(lk[:], lk[:])
                V.scalar_tensor_tensor(lk[:], lk[:], 0.95, bc(c025, SW),
                                       AL.mult, AL.add)
                xcB = gpdm1f
                G.tensor_scalar(xcB[:], xc1[:], float(BOUND), None, AL.add)
                d1 = ftile('d1')
                V.scalar_tensor_tensor(d1[:], idxc[:], float(-2 * BOUND * MB),
                                       xcB[:], AL.mult, AL.add)
                t1 = gpdm1f
                G.tensor_tensor(t1[:], rw[:], gcpf[:], AL.mult)
                dx = d1
                V.tensor_tensor(dx[:], d1[:], t1[:], AL.subtract)
                wk = ftile('wk')
                V.tensor_tensor(wk[:], rw[:], g4f[:, :, 0], AL.mult)
                V.tensor_scalar(wk[:], wk[:], float(2 * BOUND * MB), None,
                                AL.add)
                t3 = gchpf
                G.tensor_tensor(t3[:], rh[:], gchpf[:], AL.mult)
                ya = gchpf
                V.scalar_tensor_tensor(ya[:], idxc[:], float(2 * BOUND * MB),
                                       t3[:], AL.mult, AL.add)
                V.tensor_scalar(ya[:], ya[:], float(BOUND), None, AL.subtract)
                hk = ftile('hk')
                G.tensor_tensor(hk[:], rh[:], g4f[:, :, 1], AL.mult)
                G.tensor_scalar(hk[:], hk[:], float(2 * BOUND * MB), None,
                                AL.add)
                yb = gcpf
                G.tensor_tensor(yb[:], ya[:], hk[:], AL.add)
                lkwk = sfu
                V.tensor_tensor(lkwk[:], lk[:], wk[:], AL.mult)
                dxl = sfu
                V.tensor_tensor(dxl[:], lkwk[:], dx[:], AL.subtract)
                omlk = shf
                V.tensor_scalar(omlk[:], lk[:], -1.0, 1.0, AL.mult, AL.add)
                wbdk1 = lndk1
                G.tensor_tensor(wbdk1[:], wb[:], dk1[:], AL.mult)
                lkdk = e2
                V.tensor_tensor(lkdk[:], lk[:], dk[:], AL.mult)
                wcn = lndk1
                G.tensor_tensor(wcn[:], omlk[:], wbdk1[:], AL.mult)
                G.tensor_tensor(wcn[:], wcn[:], lkdk[:], AL.add)
                Wt = lndk1
                V.tensor_tensor(Wt[:], wcn[:], wk[:], AL.mult)
                lkwb = e1
                G.tensor_tensor(lkwb[:], lk[:], wb[:], AL.mult)
                ycn = ftile('ycn')
                V.tensor_tensor(ycn[:], lkwb[:], yb[:], AL.mult)
                t6 = sc0
                G.tensor_tensor(t6[:], omlk[:], ya[:], AL.mult)
                V.tensor_tensor(ycn[:], ycn[:], t6[:], AL.add)
                ycd = shf
                G.tensor_tensor(ycd[:], omlk[:], lkwb[:], AL.add)
                hkdxl = sc0
                V.tensor_tensor(hkdxl[:], hk[:], dxl[:], AL.mult)
                Wdx = e2
                G.tensor_tensor(Wdx[:], Wt[:], dx[:], AL.mult)
                t7 = es
                V.tensor_tensor(t7[:], ycd[:], ya[:], AL.mult)
                numl = es
                V.tensor_tensor(numl[:], t7[:], hkdxl[:], AL.mult)
                t8 = e1
                G.tensor_tensor(t8[:], Wdx[:], ycn[:], AL.mult)
                V.tensor_tensor(numl[:], numl[:], t8[:], AL.add)
                denl = sc0
                G.tensor_tensor(denl[:], hkdxl[:], Wdx[:], AL.add)
                G.tensor_tensor(denl[:], denl[:], ycd[:], AL.mult)
                dxr = wk
                V.tensor_tensor(dxr[:], wk[:], dx[:], AL.subtract)
                Wdxr = e2
                V.tensor_tensor(Wdxr[:], Wt[:], dxr[:], AL.mult)
                numr = ycn
                V.tensor_tensor(numr[:], numr[:], Wdxr[:], AL.mult)
                wbyb = gcpf
                G.tensor_tensor(wbyb[:], wb[:], yb[:], AL.mult)
                t9 = gcpf
                G.tensor_tensor(t9[:], wbyb[:], hk[:], AL.mult)
                G.tensor_tensor(t9[:], t9[:], dxl[:], AL.mult)
                G.tensor_tensor(t9[:], t9[:], ycd[:], AL.mult)
                V.tensor_tensor(numr[:], numr[:], t9[:], AL.subtract)
                wbhk = lndk
                G.tensor_tensor(wbhk[:], wb[:], hk[:], AL.mult)
                G.tensor_tensor(wbhk[:], wbhk[:], dxl[:], AL.mult)
                denr = gchpf
                V.tensor_tensor(denr[:], Wdxr[:], wbhk[:], AL.subtract)
                V.tensor_tensor(denr[:], denr[:], ycd[:], AL.mult)
                leftm = ftile('leftm', U8)
                V.tensor_scalar(leftm[:], dxl[:], 0.0, None, AL.is_ge)
                V.copy_predicated(numr[:], leftm[:], numl[:])
                V.copy_predicated(denr[:], leftm[:], denl[:])
                iden = lndk1
                V.reciprocal(iden[:], denr[:])
                num0 = ftile('num0')
                G.tensor_tensor(num0[:], g3f[:, :, 1], xc0[:], AL.mult)
                G.tensor_tensor(num0[:], num0[:], g3f[:, :, 0], AL.add)
                den0 = ftile('den0')
                G.tensor_tensor(den0[:], g3f[:, :, 2], xc0[:], AL.mult)
                G.tensor_scalar(den0[:], den0[:], 1.0, None, AL.add)
                iden0 = den0
                V.reciprocal(iden0[:], den0[:])
                V.tensor_tensor(x1w, numr[:], iden[:], AL.mult)
                G.tensor_tensor(x0w, num0[:], iden0[:], AL.mult)
                ab1 = gpdm1f
                A.activation(ab1[:], x1r[:], AF.Abs)
                out1 = ftile('out1', U8)
                V.tensor_scalar(out1[:], ab1[:], float(BOUND), None, AL.is_gt)
                V.copy_predicated(x1w, out1[:], x1r[:])
                ab0 = gpdm1f
                A.activation(ab0[:], x0r[:], AF.Abs)
                out0 = ftile('out0', U8)
                V.tensor_scalar(out0[:], ab0[:], float(BOUND), None, AL.is_gt)
                V.copy_predicated(x0w, out0[:], x0r[:])

        nc.sync.dma_start(yr[:], yout[:])

    nc.compile()
    return nc


def make_const_inputs(pc, cchunk):
    """Host-side replicated constant block matching crep layout."""
    CC = cchunk
    n = 16 + 32 + 32 + 96 + 512 + CC * 16
    crep = np.zeros((128, n), dtype=f32)
    crep[:, 0:16] = np.arange(1, 17, dtype=f32)
    crep[:, 16:48] = np.arange(32, dtype=f32)
    crep[:, 48:80] = pc['bnd32']
    crep[:, 80:112] = pc['tabA']
    crep[:, 112:144] = pc['tabB']
    crep[:, 144:176] = pc['tabD']
    b2rep = np.zeros(512, dtype=f32)
    for b in range(8):
        b2rep[63 * b:63 * b + 63] = pc['b2odd']
    crep[:, 176:688] = b2rep
    msk = np.ones((CC, 16), dtype=f32)
    msk[:, 0] = 0.0
    crep[:, 688:688 + CC * 16] = msk.reshape(-1)
    return crep


_CACHE = {}


def kernel(z, W0, b0, W1, b1, W2, b2):
    from concourse.bass_utils import run_bass_kernel_spmd
    pc = precompute(W0, b0, W1, b1, W2, b2)
    n = z.shape[0]
    npc = n // N_CORES
    ncols = npc // 128
    key = ('prog', ncols, pc['fast_hyper'], pc['b2zero'], pc['uniform32'])
    if key not in _CACHE:
        _CACHE[key] = build_program(pc, ncols=ncols)
    nc = _CACHE[key]
    crep = make_const_inputs(pc, 64)
    base = dict(qb=pc['qb'], u2=pc['u2'], w2t=pc['w2t'], b1c=pc['b1c'],
                crep=crep)
    in_maps = []
    for i in range(N_CORES):
        m = dict(base)
        m['z'] = np.ascontiguousarray(z[i * npc:(i + 1) * npc])
        in_maps.append(m)
    res = run_bass_kernel_spmd(nc, in_maps, list(range(N_CORES)))
    out = np.concatenate([res.results[i]['y'] for i in range(N_CORES)], axis=0)
    return out.astype(z.dtype)


def kernel_profiled(z, W0, b0, W1, b1, W2, b2, trace_dir=None):
    """Run with NTFF profiling; returns exec_time_ns (or None)."""
    from concourse.bass_utils import run_bass_kernel_spmd
    pc = precompute(W0, b0, W1, b1, W2, b2)
    n = z.shape[0]
    npc = n // N_CORES
    ncols = npc // 128
    key = ('prog', ncols, pc['fast_hyper'], pc['b2zero'], pc['uniform32'])
    if key not in _CACHE:
        _CACHE[key] = build_program(pc, ncols=ncols)
    nc = _CACHE[key]
    crep = make_const_inputs(pc, 64)
    base = dict(qb=pc['qb'], u2=pc['u2'], w2t=pc['w2t'], b1c=pc['b1c'],
                crep=crep)
    in_maps = []
    for i in range(N_CORES):
        m = dict(base)
        m['z'] = np.ascontiguousarray(z[i * npc:(i + 1) * npc])
        in_maps.append(m)
    import tempfile, shutil
    td = trace_dir or tempfile.mkdtemp(prefix='ktrace_')
    if os.path.isdir(td):
        shutil.rmtree(td, ignore_errors=True)
    os.makedirs(td, exist_ok=True)
    res = run_bass_kernel_spmd(nc, in_maps, list(range(N_CORES)),
                               trace=True, tmpdir=td)
    return res.exec_time_ns

